# revision 1
# baseline (speedup 1.0000x reference)
"""Trainium2 Bass kernel for a 2-layer GCN encoder + MLP head (PyG GCNConv).

Strategy (8 NeuronCores, node-parallel):
  - Nodes sharded by contiguous range: core q owns rows [q*SH, (q+1)*SH).
  - conv linear (x @ Wc1) computed shard-local on PE (bf16 inputs, fp32 PSUM).
  - z0 shards AllGather'ed to a full replicated table Z0 [NP, 256] (bf16).
  - Aggregation out[d] = sum_e norm_e * z[src_e] done per destination tile:
    edges grouped (host-side bin-packing) into T tiles of <=128 dst nodes and
    <=C*128 edges; per tile one dma_gather fetches all edge source rows, a
    fused DVE tensor_scalar builds the norm-scaled one-hot S_T [128e,128d]
    per 128-edge chunk, and PE matmuls accumulate S_T.T @ msg in PSUM.
  - dma_gather indices are int16 (max 32767) so the Z table is split in two
    halves and each tile issues two gathers (lo/hi src).
  - h1 tiles are bias+relu'ed, transposed on PE, pushed through Wc2, and the
    z1 rows are indirect-DMA-scattered back to the shard layout; AllGather;
    layer-2 aggregation likewise produces h2; AllGather; the MLP head runs
    on B/8 variants per core (feature-major matmuls, ACT bias+relu).
All heavy compute is bf16 with fp32 PSUM accumulation.
"""
import sys

for _p in ("/opt/trn_rl_repo",):
    if _p not in sys.path:
        sys.path.insert(0, _p)

import numpy as np
import ml_dtypes

bf16 = ml_dtypes.bfloat16

P = 128
H = 256          # gcn hidden width (fixed)
HH = 128         # head hidden width (fixed)
OH = 40          # wt_onehot + mut_onehot width (fixed)
NCORES = 8


class Cfg:
    def __init__(self, N, E, D_IN, B):
        self.N, self.E, self.D_IN, self.B = N, E, D_IN, B
        assert N % NCORES == 0
        self.SH = N // NCORES                      # real rows per shard
        shp = -(-self.SH // P) * P
        if shp == self.SH:
            shp += P                               # need >=1 dump row
        self.SHP = shp                             # padded rows per shard
        self.NP = NCORES * self.SHP                # padded global rows
        assert self.NP % 2 == 0
        self.NPH = self.NP // 2                    # half-table rows (int16 idx)
        assert self.NPH < 32768
        self.KT = -(-D_IN // P)                    # k tiles for conv1
        self.KPAD = self.KT * P
        self.MT = self.SHP // P                    # m tiles per shard
        self.BPC = self.B // NCORES                # batch per core
        assert self.BPC % P == 0
        self.BCH = self.BPC // P                   # batch chunks


REAL = Cfg(N=50000, E=800000, D_IN=1281, B=4096)


# ---------------------------------------------------------------- host prep

def _pack_idx16(seq):
    """idx sequence [n] -> wrapped-16 + replicated layout [128, n//16] int16."""
    n = seq.shape[0]
    assert n % 16 == 0
    a = seq.reshape(n // 16, 16).T.astype(np.int16)
    return np.tile(a, (8, 1))


def _pack_core(cfg, cl, ch, d_loc, srcp, nv, count_only=False):
    """Bin-pack one core's edges into tiles (<=128 nodes, <=cl*128 lo edges,
    <=ch*128 hi edges). Returns tiles as (nodes, lo_lists, hi_lists) or count."""
    SH = cfg.SH
    order = np.argsort(d_loc, kind="stable")
    d_s = d_loc[order]
    counts = np.bincount(d_s, minlength=SH)
    starts = np.zeros(SH + 1, np.int64)
    np.cumsum(counts, out=starts[1:])
    lo_mask = srcp[order] < cfg.NPH
    # per-node lo/hi counts
    klo = np.zeros(SH, np.int64)
    np.add.at(klo, d_s[lo_mask], 1)
    ktot = counts
    khi = ktot - klo

    node_order = np.argsort(-ktot, kind="stable")
    cap_l, cap_h = cl * P, ch * P
    tiles = []  # [n_nodes, lo_cnt, hi_cnt, node_list]
    for r in node_order:
        kl, kh = klo[r], khi[r]
        placed = False
        for t in tiles:
            if t[0] < P and t[1] + kl <= cap_l and t[2] + kh <= cap_h:
                t[0] += 1
                t[1] += kl
                t[2] += kh
                t[3].append(r)
                placed = True
                break
        if not placed:
            tiles.append([1, kl, kh, [r]])
    if count_only:
        return len(tiles)
    return tiles, order, starts, lo_mask


def _build_core_arrays(cfg, q, T, cl, ch, tiles, order, starts, lo_mask,
                       srcp, nv):
    """Build gidx/dsel/nrm/scat arrays for one core."""
    C = cl + ch
    n_slots = T * C * P
    gidx_seq = np.zeros(n_slots, np.int64)
    dsel = np.zeros((P, T * C), np.float32)
    nrm = np.zeros((P, T * C), np.float32)
    scat = np.full((P, T), cfg.SH, np.int32)  # dump row default

    for t, tl in enumerate(tiles):
        lo_idx, lo_d, lo_n = [], [], []
        hi_idx, hi_d, hi_n = [], [], []
        for d, r in enumerate(tl[3]):
            scat[d, t] = r
            es = order[starts[r]:starts[r + 1]]
            lm = lo_mask[starts[r]:starts[r + 1]]
            sp = srcp[es]
            nn = nv[es]
            lo_idx.extend(sp[lm].tolist())
            lo_d.extend([d] * int(lm.sum()))
            lo_n.extend(nn[lm].tolist())
            hm = ~lm
            hi_idx.extend((sp[hm] - cfg.NPH).tolist())
            hi_d.extend([d] * int(hm.sum()))
            hi_n.extend(nn[hm].tolist())
        npad_l = cl * P - len(lo_idx)
        npad_h = ch * P - len(hi_idx)
        assert npad_l >= 0 and npad_h >= 0
        seq_idx = lo_idx + [0] * npad_l + hi_idx + [0] * npad_h
        seq_d = lo_d + [0] * npad_l + hi_d + [0] * npad_h
        seq_n = lo_n + [0.0] * npad_l + hi_n + [0.0] * npad_h
        base = t * C * P
        gidx_seq[base:base + C * P] = seq_idx
        a_d = np.asarray(seq_d, np.float32).reshape(C, P).T
        a_n = np.asarray(seq_n, np.float32).reshape(C, P).T
        dsel[:, t * C:(t + 1) * C] = a_d
        nrm[:, t * C:(t + 1) * C] = a_n

    # pack idx per (tile, half) segment
    cols = []
    for t in range(T):
        base = t * C * P
        cols.append(_pack_idx16(gidx_seq[base:base + cl * P]))
        cols.append(_pack_idx16(gidx_seq[base + cl * P:base + C * P]))
    gidx = np.concatenate(cols, axis=1)  # [128, T*C*8]
    return gidx, dsel, nrm, scat


def host_prep(cfg, x, wt_onehot, mut_onehot, Wc1, bc1, Wc2, bc2,
              Wh1, bh1, Wh2, bh2, Wh3, bh3, edge_index, var_node_idx):
    N, E, SH, SHP = cfg.N, cfg.E, cfg.SH, cfg.SHP
    src = np.asarray(edge_index[0], np.int64)
    dst = np.asarray(edge_index[1], np.int64)
    loop = np.arange(N, dtype=np.int64)
    src_all = np.concatenate([src, loop])
    dst_all = np.concatenate([dst, loop])
    deg = np.bincount(dst_all, minlength=N).astype(np.float32)
    dinv = np.where(deg > 0, 1.0 / np.sqrt(np.maximum(deg, 1.0)), 0.0).astype(np.float32)
    norm = (dinv[src_all] * dinv[dst_all]).astype(np.float32)
    srcp_all = (src_all // SH) * SHP + (src_all % SH)

    core_of = dst_all // SH
    per_core = []
    for q in range(NCORES):
        m = core_of == q
        per_core.append((dst_all[m] - q * SH, srcp_all[m], norm[m]))

    # choose caps
    avg_deg = (E + N) / N
    base = max(1, int(np.ceil(avg_deg * P / 2 / P)))
    cands = [(base, base), (base + 1, base + 1), (base, base + 1),
             (base + 1, base), (base + 2, base + 2)]
    best = None
    packs_cache = {}
    for (cl, ch) in cands:
        Ts = []
        packs = []
        for q in range(NCORES):
            d_loc, srcp, nv = per_core[q]
            pk = _pack_core(cfg, cl, ch, d_loc, srcp, nv)
            packs.append(pk)
            Ts.append(len(pk[0]))
        T_need = max(Ts)
        cost = T_need * (cl + ch)
        if best is None or cost < best[0]:
            best = (cost, cl, ch, T_need)
            packs_cache = {q: packs[q] for q in range(NCORES)}
    _, cl, ch, T = best
    C = cl + ch

    # shared weights
    wc1 = np.zeros((cfg.KPAD, H), bf16)
    wc1[:cfg.D_IN] = np.asarray(Wc1, np.float32).astype(bf16)
    wc2 = np.asarray(Wc2, np.float32).astype(bf16)
    wh1 = np.zeros((3 * P, HH), bf16)
    wh1[:H + OH] = np.asarray(Wh1, np.float32).astype(bf16)
    wh2 = np.asarray(Wh2, np.float32).astype(bf16)
    wh3 = np.asarray(Wh3, np.float32).astype(bf16)
    bb1 = np.tile(np.asarray(bc1, np.float32)[None, :], (P, 1))
    bb2 = np.tile(np.asarray(bc2, np.float32)[None, :], (P, 1))
    bh1v = np.asarray(bh1, np.float32).reshape(HH, 1)
    bh2v = np.asarray(bh2, np.float32).reshape(HH // 2, 1)
    bh3v = np.asarray(bh3, np.float32).reshape(1, 1)

    x = np.asarray(x, np.float32)
    wt_b = np.asarray(wt_onehot, np.float32).astype(bf16)
    mut_b = np.asarray(mut_onehot, np.float32).astype(bf16)
    vni = np.asarray(var_node_idx, np.int64)
    vrow = (vni // SH) * SHP + (vni % SH)

    in_maps = []
    meta = dict(T=T, cl=cl, ch=ch)
    for q in range(NCORES):
        d_loc, srcp, nv = per_core[q]
        tiles, order, starts, lo_mask = packs_cache[q]
        gidx, dsel, nrm, scat = _build_core_arrays(
            cfg, q, T, cl, ch, tiles, order, starts, lo_mask, srcp, nv)
        xT = np.zeros((cfg.KPAD, SHP), bf16)
        xT[:cfg.D_IN, :SH] = x[q * SH:(q + 1) * SH].T.astype(bf16)
        vr = vrow[q * cfg.BPC:(q + 1) * cfg.BPC]
        vidx = vr.reshape(cfg.BCH, P).T.astype(np.int32)
        ohT = np.concatenate(
            [wt_b[q * cfg.BPC:(q + 1) * cfg.BPC].T,
             mut_b[q * cfg.BPC:(q + 1) * cfg.BPC].T], axis=0)  # [40, BPC]
        in_maps.append(dict(
            xT=xT, gidx=gidx, dsel=dsel, nrm=nrm, scat=scat,
            vidx=np.ascontiguousarray(vidx), ohT=np.ascontiguousarray(ohT),
            wc1=wc1, wc2=wc2, wh1=wh1, wh2=wh2, wh3=wh3,
            bb1=bb1, bb2=bb2, bh1v=bh1v, bh2v=bh2v, bh3v=bh3v,
        ))
    return in_maps, meta


# ------------------------------------------------------------- bass program

def build_program(cfg, T, cl, ch):
    import concourse.bass as bass
    import concourse.mybir as mybir
    import concourse.tile as tile
    from concourse import bacc
    from concourse.masks import make_identity

    C = cl + ch
    nc = bacc.Bacc("TRN2", target_bir_lowering=False, debug=False,
                   num_devices=NCORES)
    f32, bfl, i16, i32 = (mybir.dt.float32, mybir.dt.bfloat16,
                          mybir.dt.int16, mybir.dt.int32)

    # I/O
    xT = nc.dram_tensor("xT", [cfg.KPAD, cfg.SHP], bfl, kind="ExternalInput")
    gidx = nc.dram_tensor("gidx", [P, T * C * 8], i16, kind="ExternalInput")
    dsel = nc.dram_tensor("dsel", [P, T * C], f32, kind="ExternalInput")
    nrm = nc.dram_tensor("nrm", [P, T * C], f32, kind="ExternalInput")
    scat = nc.dram_tensor("scat", [P, T], i32, kind="ExternalInput")
    vidx = nc.dram_tensor("vidx", [P, cfg.BCH], i32, kind="ExternalInput")
    ohT = nc.dram_tensor("ohT", [OH, cfg.BPC], bfl, kind="ExternalInput")
    wc1 = nc.dram_tensor("wc1", [cfg.KPAD, H], bfl, kind="ExternalInput")
    wc2 = nc.dram_tensor("wc2", [H, H], bfl, kind="ExternalInput")
    wh1 = nc.dram_tensor("wh1", [3 * P, HH], bfl, kind="ExternalInput")
    wh2 = nc.dram_tensor("wh2", [HH, HH // 2], bfl, kind="ExternalInput")
    wh3 = nc.dram_tensor("wh3", [HH // 2, 1], bfl, kind="ExternalInput")
    bb1 = nc.dram_tensor("bb1", [P, H], f32, kind="ExternalInput")
    bb2 = nc.dram_tensor("bb2", [P, H], f32, kind="ExternalInput")
    bh1v = nc.dram_tensor("bh1v", [HH, 1], f32, kind="ExternalInput")
    bh2v = nc.dram_tensor("bh2v", [HH // 2, 1], f32, kind="ExternalInput")
    bh3v = nc.dram_tensor("bh3v", [1, 1], f32, kind="ExternalInput")
    out = nc.dram_tensor("out", [1, cfg.BPC], f32, kind="ExternalOutput")

    # internal DRAM
    z0in = nc.dram_tensor("z0in", [cfg.SHP, H], bfl, kind="Internal")
    z1in = nc.dram_tensor("z1in", [cfg.SHP, H], bfl, kind="Internal")
    h2in = nc.dram_tensor("h2in", [cfg.SHP, H], bfl, kind="Internal")
    Z0 = nc.dram_tensor("Z0", [cfg.NP, H], bfl, kind="Internal",
                        addr_space="Shared")
    Z1 = nc.dram_tensor("Z1", [cfg.NP, H], bfl, kind="Internal",
                        addr_space="Shared")
    H2 = nc.dram_tensor("H2", [cfg.NP, H], bfl, kind="Internal",
                        addr_space="Shared")
    rg = [list(range(NCORES))]

    with tile.TileContext(nc) as tc:
        with tc.tile_pool(name="const", bufs=1) as const:
            iota_i = const.tile([P, P], i32)
            nc.gpsimd.iota(iota_i[:], pattern=[[1, P]], base=0,
                           channel_multiplier=0)
            iota_b = const.tile([P, P], bfl)
            nc.vector.tensor_copy(iota_b[:], iota_i[:])
            ident = const.tile([P, P], bfl)
            make_identity(nc, ident[:])

            def load(ap, shape, dt):
                t = const.tile(shape, dt, tag=ap.tensor.name)
                nc.sync.dma_start(t[:], ap)
                return t

            wc1_sb = load(wc1.rearrange("(t p) n -> p t n", p=P)[:],
                          [P, cfg.KT, H], bfl)
            wc2_sb = load(wc2.rearrange("(t p) n -> p t n", p=P)[:],
                          [P, 2, H], bfl)
            wh1_sb = load(wh1.rearrange("(t p) n -> p t n", p=P)[:],
                          [P, 3, HH], bfl)
            wh2_sb = load(wh2[:], [HH, HH // 2], bfl)
            wh3_sb = load(wh3[:], [HH // 2, 1], bfl)
            bb1_sb = load(bb1[:], [P, H], f32)
            bb2_sb = load(bb2[:], [P, H], f32)
            bh1_sb = load(bh1v[:], [HH, 1], f32)
            bh2_sb = load(bh2v[:], [HH // 2, 1], f32)
            bh3_sb = load(bh3v[:], [1, 1], f32)
            gidx_sb = load(gidx[:], [P, T * C * 8], i16)
            dsel_sb = load(dsel[:], [P, T * C], f32)
            nrm_sb = load(nrm[:], [P, T * C], f32)
            scat_sb = load(scat[:], [P, T], i32)
            vidx_sb = load(vidx[:], [P, cfg.BCH], i32)
            ohT_sb = load(ohT[:], [OH, cfg.BPC], bfl)

            # zero the dump rows of the scatter targets
            zpad = const.tile([P, H], bfl)
            nc.any.memset(zpad[:], 0.0)
            npad = cfg.SHP - cfg.SH
            nc.sync.dma_start(z1in[cfg.SH:cfg.SHP, :], zpad[:npad, :])
            nc.sync.dma_start(h2in[cfg.SH:cfg.SHP, :], zpad[:npad, :])

            # ---------------- phase A: conv1 linear z0 = x @ Wc1
            MBS = 7
            with tc.tile_pool(name="c1sb", bufs=3) as c1sb, \
                 tc.tile_pool(name="c1ev", bufs=3) as c1ev, \
                 tc.tile_pool(name="c1ps", bufs=MBS + 1, space="PSUM") as c1ps:
                for mb0 in range(0, cfg.MT, MBS):
                    mbn = min(MBS, cfg.MT - mb0)
                    accs = [c1ps.tile([P, H], f32, tag="convacc",
                                      name=f"convacc_{mb0}_{j}")
                            for j in range(mbn)]
                    for kt in range(cfg.KT):
                        slab = c1sb.tile([P, MBS * P], bfl, tag="slab")
                        nc.sync.dma_start(
                            slab[:, :mbn * P],
                            xT[kt * P:(kt + 1) * P, mb0 * P:(mb0 + mbn) * P])
                        for j in range(mbn):
                            nc.tensor.matmul(
                                accs[j][:], lhsT=slab[:, j * P:(j + 1) * P],
                                rhs=wc1_sb[:, kt, :],
                                start=(kt == 0), stop=(kt == cfg.KT - 1))
                    for j in range(mbn):
                        zb = c1ev.tile([P, H], bfl, tag="zev")
                        nc.vector.tensor_copy(zb[:], accs[j][:])
                        r0 = (mb0 + j) * P
                        nc.sync.dma_start(z0in[r0:r0 + P, :], zb[:])

            nc.gpsimd.collective_compute(
                "AllGather", mybir.AluOpType.bypass, replica_groups=rg,
                ins=[z0in[:]], outs=[Z0[:]])

            # ---------------- aggregation layers
            def agg_layer(Z, bias_sb, out_dram, do_conv2):
                with tc.tile_pool(name="agsb", bufs=3) as agsb, \
                     tc.tile_pool(name="agps", bufs=2, space="PSUM") as agps:
                    Zlo = Z[:cfg.NPH, :]
                    Zhi = Z[cfg.NPH:, :]
                    for t in range(T):
                        msg = agsb.tile([P, C, H], bfl, tag="msg")
                        off = t * C * 8
                        nc.gpsimd.dma_gather(
                            msg[:, :cl, :], Zlo, gidx_sb[:, off:off + cl * 8],
                            cl * P, cl * P, H, single_packet=False)
                        nc.gpsimd.dma_gather(
                            msg[:, cl:, :], Zhi,
                            gidx_sb[:, off + cl * 8:off + C * 8],
                            ch * P, ch * P, H, single_packet=False)
                        acc = agps.tile([P, H], f32, tag="agacc")
                        for c in range(C):
                            st = agsb.tile([P, P], bfl, tag="st")
                            col = t * C + c
                            nc.vector.tensor_scalar(
                                out=st[:], in0=iota_b[:],
                                scalar1=dsel_sb[:, col:col + 1],
                                scalar2=nrm_sb[:, col:col + 1],
                                op0=mybir.AluOpType.is_equal,
                                op1=mybir.AluOpType.mult)
                            nc.tensor.matmul(acc[:], lhsT=st[:],
                                             rhs=msg[:, c, :],
                                             start=(c == 0), stop=(c == C - 1))
                        hf = agsb.tile([P, H], f32, tag="hf")
                        nc.vector.tensor_tensor(out=hf[:], in0=acc[:],
                                                in1=bias_sb[:],
                                                op=mybir.AluOpType.add)
                        hb = agsb.tile([P, H], bfl, tag="hb")
                        nc.vector.tensor_scalar_max(hb[:], hf[:], 0.0)
                        if do_conv2:
                            ht = agsb.tile([P, H], bfl, tag="ht")
                            for k in range(2):
                                pt = agps.tile([P, P], bfl, space="PSUM",
                                               tag="pt")
                                nc.tensor.transpose(
                                    pt[:], hb[:, k * P:(k + 1) * P], ident[:])
                                nc.vector.tensor_copy(
                                    ht[:, k * P:(k + 1) * P], pt[:])
                            pz = agps.tile([P, H], f32, tag="pz")
                            for k in range(2):
                                nc.tensor.matmul(
                                    pz[:], lhsT=ht[:, k * P:(k + 1) * P],
                                    rhs=wc2_sb[:, k, :],
                                    start=(k == 0), stop=(k == 1))
                            res = agsb.tile([P, H], bfl, tag="res")
                            nc.vector.tensor_copy(res[:], pz[:])
                        else:
                            res = hb
                        nc.gpsimd.indirect_dma_start(
                            out=out_dram[:],
                            out_offset=bass.IndirectOffsetOnAxis(
                                ap=scat_sb[:, t:t + 1], axis=0),
                            in_=res[:], in_offset=None)

            agg_layer(Z0, bb1_sb, z1in, do_conv2=True)
            nc.gpsimd.collective_compute(
                "AllGather", mybir.AluOpType.bypass, replica_groups=rg,
                ins=[z1in[:]], outs=[Z1[:]])
            agg_layer(Z1, bb2_sb, h2in, do_conv2=False)
            nc.gpsimd.collective_compute(
                "AllGather", mybir.AluOpType.bypass, replica_groups=rg,
                ins=[h2in[:]], outs=[H2[:]])

            # ---------------- head
            with tc.tile_pool(name="hdsb", bufs=2) as hdsb, \
                 tc.tile_pool(name="hdps", bufs=2, space="PSUM") as hdps:
                zt0 = hdsb.tile([P, cfg.BPC], bfl, tag="zt0")
                zt1 = hdsb.tile([P, cfg.BPC], bfl, tag="zt1")
                for j in range(cfg.BCH):
                    g = hdsb.tile([P, H], bfl, tag="hg")
                    nc.gpsimd.indirect_dma_start(
                        out=g[:], out_offset=None, in_=H2[:],
                        in_offset=bass.IndirectOffsetOnAxis(
                            ap=vidx_sb[:, j:j + 1], axis=0))
                    for k in range(2):
                        pt = hdps.tile([P, P], bfl, space="PSUM", tag="hpt")
                        nc.tensor.transpose(pt[:], g[:, k * P:(k + 1) * P],
                                            ident[:])
                        dstt = zt0 if k == 0 else zt1
                        nc.vector.tensor_copy(
                            dstt[:, j * P:(j + 1) * P], pt[:])
                ph1 = hdps.tile([P, cfg.BPC], f32, tag="ph1")
                nc.tensor.matmul(ph1[:], lhsT=wh1_sb[:, 0, :], rhs=zt0[:],
                                 start=True, stop=False)
                nc.tensor.matmul(ph1[:], lhsT=wh1_sb[:, 1, :], rhs=zt1[:],
                                 start=False, stop=False)
                nc.tensor.matmul(ph1[:], lhsT=wh1_sb[:OH, 2, :],
                                 rhs=ohT_sb[:], start=False, stop=True)
                a1 = hdsb.tile([P, cfg.BPC], bfl, tag="a1")
                nc.scalar.activation(a1[:], ph1[:],
                                     mybir.ActivationFunctionType.Relu,
                                     bias=bh1_sb[:])
                ph2 = hdps.tile([HH // 2, cfg.BPC], f32, tag="ph2")
                nc.tensor.matmul(ph2[:], lhsT=wh2_sb[:], rhs=a1[:],
                                 start=True, stop=True)
                a2 = hdsb.tile([HH // 2, cfg.BPC], bfl, tag="a2")
                nc.scalar.activation(a2[:], ph2[:],
                                     mybir.ActivationFunctionType.Relu,
                                     bias=bh2_sb[:])
                ph3 = hdps.tile([1, cfg.BPC], f32, tag="ph3")
                nc.tensor.matmul(ph3[:], lhsT=wh3_sb[:], rhs=a2[:],
                                 start=True, stop=True)
                osb = hdsb.tile([1, cfg.BPC], f32, tag="osb")
                nc.vector.tensor_scalar_add(osb[:], ph3[:], bh3_sb[:, :1])
                nc.sync.dma_start(out[:], osb[:])

    nc.compile()
    return nc


# ------------------------------------------------------------------ driver

_CACHE = {}


def _get_program(cfg, T, cl, ch):
    key = (cfg.N, cfg.E, cfg.D_IN, cfg.B, T, cl, ch)
    if key not in _CACHE:
        _CACHE[key] = build_program(cfg, T, cl, ch)
    return _CACHE[key]


def kernel(**inputs):
    cfg = REAL
    in_maps, meta = host_prep(cfg, **inputs)
    nc = _get_program(cfg, meta["T"], meta["cl"], meta["ch"])
    from concourse import bass_utils
    res = bass_utils.run_bass_kernel_spmd(
        nc, in_maps, core_ids=list(range(NCORES)))
    outs = [np.asarray(res.results[q]["out"]).reshape(cfg.BPC)
            for q in range(NCORES)]
    return np.concatenate(outs).astype(np.float32)



# revision 5
# speedup vs baseline: 2.3156x; 2.3156x over previous
"""Trainium2 Bass kernel for a 2-layer GCN encoder + MLP head (PyG GCNConv).

Strategy (8 NeuronCores, node-parallel), v2:
  - Nodes sharded by contiguous range: core q owns rows [q*SH, (q+1)*SH).
  - conv1 linear (x @ Wc1) computed shard-local on PE (bf16, fp32 PSUM),
    z0 shards AllGather'ed to a replicated table Z0 [NP, 256] bf16.
  - Layer-2 pruning: the head only reads h2 at var_node_idx, so layer 2
    aggregates only over in-edges of the ~4k distinct variant nodes
    (owner-core sharded); no H2 AllGather, the head runs on the owner core
    and the host re-permutes the (value, b-position) pairs.
  - Layer-1 pruning: h1 is only needed at U1 = variants + their in-edge
    sources (~75% of nodes), so layer-1 tiles cover only owned U1 nodes.
  - Aggregation per dst tile: one dma_gather per (tile, half-table) fetches
    edge source rows (SWDGE queues alternate per tile to overlap descriptor
    generation), and PE matmuls accumulate st.T @ msg in PSUM where the
    norm-scaled one-hot st chunks are PREBUILT ON HOST and DMA-loaded
    (frees DVE from 2k tensor_scalar builds which contend with SWDGE).
  - dma_gather indices are int16 so the Z table is split lo/hi.
  - conv2 fused into layer-1 tile epilogue (PE transpose + matmul).
All heavy compute is bf16 with fp32 PSUM accumulation.
"""
import sys

for _p in ("/opt/trn_rl_repo",):
    if _p not in sys.path:
        sys.path.insert(0, _p)

import numpy as np
import ml_dtypes

bf16 = ml_dtypes.bfloat16

P = 128
H = 256          # gcn hidden width (fixed)
HH = 128         # head hidden width (fixed)
OH = 40          # wt_onehot + mut_onehot width (fixed)
NCORES = 8


class Cfg:
    def __init__(self, N, E, D_IN, B):
        self.N, self.E, self.D_IN, self.B = N, E, D_IN, B
        assert N % NCORES == 0
        self.SH = N // NCORES                      # real rows per shard
        shp = -(-self.SH // P) * P
        if shp == self.SH:
            shp += P                               # need >=1 dump row
        self.SHP = shp                             # padded rows per shard
        self.NP = NCORES * self.SHP                # padded global rows
        assert self.NP % 2 == 0
        self.NPH = self.NP // 2                    # half-table rows (int16 idx)
        assert self.NPH < 32768
        self.KT = -(-D_IN // P)                    # k tiles for conv1
        self.KPAD = self.KT * P
        self.MT = self.SHP // P                    # m tiles per shard
        self.B = B


REAL = Cfg(N=50000, E=800000, D_IN=1281, B=4096)


# ---------------------------------------------------------------- host prep

def _pack_idx16(seq):
    """idx sequence [n] -> wrapped-16 + replicated layout [128, n//16] int16."""
    n = seq.shape[0]
    assert n % 16 == 0
    a = seq.reshape(n // 16, 16).T.astype(np.int16)
    return np.tile(a, (8, 1))


def _pack_core(nloc, cl, ch, d_loc, srcp, nv, nph):
    """Bin-pack edges into tiles (<=128 dst nodes, <=cl*128 lo edges,
    <=ch*128 hi edges). d_loc in [0, nloc)."""
    order = np.argsort(d_loc, kind="stable")
    d_s = d_loc[order]
    counts = np.bincount(d_s, minlength=nloc)
    starts = np.zeros(nloc + 1, np.int64)
    np.cumsum(counts, out=starts[1:])
    lo_mask = srcp[order] < nph
    klo = np.zeros(nloc, np.int64)
    np.add.at(klo, d_s[lo_mask], 1)
    khi = counts - klo

    node_order = np.argsort(-counts, kind="stable")
    cap_l, cap_h = cl * P, ch * P
    tiles = []  # [n_nodes, lo_cnt, hi_cnt, node_list]
    for r in node_order:
        if counts[r] == 0 and klo[r] == 0:
            # nodes with no edges still need a slot if they are real dsts
            # (can't happen: self loops guarantee >=1 edge)
            continue
        kl, kh = klo[r], khi[r]
        placed = False
        for t in tiles:
            if t[0] < P and t[1] + kl <= cap_l and t[2] + kh <= cap_h:
                t[0] += 1
                t[1] += kl
                t[2] += kh
                t[3].append(r)
                placed = True
                break
        if not placed:
            tiles.append([1, kl, kh, [r]])
    return tiles, order, starts, lo_mask


def _build_core_arrays(T, cl, ch, tiles, order, starts, lo_mask, srcp, nv,
                       nph, scat_of, dump_row):
    """Build gidx/stt/scat arrays for one core and one layer.
    stt is the norm-scaled one-hot, host-prebuilt: [128, T*C*128] bf16."""
    C = cl + ch
    gidx_seq = np.zeros(T * C * P, np.int64)
    stt = np.zeros((P, T * C * P), bf16)
    scat = np.full((P, T), dump_row, np.int32)

    for t, tl in enumerate(tiles):
        lo_idx, lo_d, lo_n = [], [], []
        hi_idx, hi_d, hi_n = [], [], []
        for d, r in enumerate(tl[3]):
            scat[d, t] = scat_of(r)
            es = order[starts[r]:starts[r + 1]]
            lm = lo_mask[starts[r]:starts[r + 1]]
            sp = srcp[es]
            nn = nv[es]
            lo_idx.extend(sp[lm].tolist())
            lo_d.extend([d] * int(lm.sum()))
            lo_n.extend(nn[lm].tolist())
            hm = ~lm
            hi_idx.extend((sp[hm] - nph).tolist())
            hi_d.extend([d] * int(hm.sum()))
            hi_n.extend(nn[hm].tolist())
        npad_l = cl * P - len(lo_idx)
        npad_h = ch * P - len(hi_idx)
        assert npad_l >= 0 and npad_h >= 0
        seq_idx = lo_idx + [0] * npad_l + hi_idx + [0] * npad_h
        seq_d = np.asarray(lo_d + [-1] * npad_l + hi_d + [-1] * npad_h,
                           np.int64)
        seq_n = np.asarray(lo_n + [0.0] * npad_l + hi_n + [0.0] * npad_h,
                           np.float32)
        base = t * C * P
        gidx_seq[base:base + C * P] = seq_idx
        # slot j = c*128 + p  ->  stt[p, base + c*128 + d] = norm
        valid = seq_d >= 0
        jj = np.nonzero(valid)[0]
        pp = jj % P
        cc = jj // P
        stt[pp, base + cc * P + seq_d[jj]] = seq_n[jj].astype(bf16)

    cols = []
    for t in range(T):
        base = t * C * P
        cols.append(_pack_idx16(gidx_seq[base:base + cl * P]))
        cols.append(_pack_idx16(gidx_seq[base + cl * P:base + C * P]))
    gidx = np.concatenate(cols, axis=1)  # [128, T*C*8]
    return gidx, stt, scat


def _choose_caps_and_pack(nloc_list, d_loc_list, srcp_list, nv_list, nph,
                          avg_c):
    """Pick (cl, ch) caps minimizing T*(cl+ch) over all cores; return packs."""
    base = max(1, int(np.ceil(avg_c / 2)))
    cands = [(base, base), (base + 1, base + 1), (base, base + 1),
             (base + 1, base), (base + 2, base + 2)]
    best = None
    for (cl, ch) in cands:
        packs, Ts = [], []
        for q in range(NCORES):
            pk = _pack_core(nloc_list[q], cl, ch, d_loc_list[q],
                            srcp_list[q], nv_list[q], nph)
            packs.append(pk)
            Ts.append(len(pk[0]))
        T_need = max(Ts)
        cost = T_need * (cl + ch)
        if best is None or cost < best[0]:
            best = (cost, cl, ch, T_need, packs)
    _, cl, ch, T, packs = best
    return cl, ch, T, packs


def host_prep(cfg, x, wt_onehot, mut_onehot, Wc1, bc1, Wc2, bc2,
              Wh1, bh1, Wh2, bh2, Wh3, bh3, edge_index, var_node_idx):
    N, E, SH, SHP = cfg.N, cfg.E, cfg.SH, cfg.SHP
    src = np.asarray(edge_index[0], np.int64)
    dst = np.asarray(edge_index[1], np.int64)
    loop = np.arange(N, dtype=np.int64)
    src_all = np.concatenate([src, loop])
    dst_all = np.concatenate([dst, loop])
    deg = np.bincount(dst_all, minlength=N).astype(np.float32)
    dinv = np.where(deg > 0, 1.0 / np.sqrt(np.maximum(deg, 1.0)),
                    0.0).astype(np.float32)
    norm = (dinv[src_all] * dinv[dst_all]).astype(np.float32)
    srcp_all = (src_all // SH) * SHP + (src_all % SH)

    vni = np.asarray(var_node_idx, np.int64)
    vset = np.unique(vni)
    # U1: nodes whose h1 is needed = variant nodes + sources of their in-edges
    m2 = np.isin(dst_all, vset)
    u1 = np.unique(np.concatenate([vset, src_all[m2]]))
    u1_mask = np.zeros(N, bool)
    u1_mask[u1] = True

    # ---------------- layer-1 structures (dst in owned U1)
    m1 = u1_mask[dst_all]
    d1, s1, n1 = dst_all[m1], srcp_all[m1], norm[m1]
    core1 = d1 // SH
    d1_loc, s1_l, n1_l, nloc1 = [], [], [], []
    for q in range(NCORES):
        m = core1 == q
        d1_loc.append(d1[m] - q * SH)
        s1_l.append(s1[m])
        n1_l.append(n1[m])
        nloc1.append(SH)
    avg_c1 = (m1.sum() / NCORES) / (SH * 0.754) * P / P  # rough
    avg_c1 = max(2.0, (m1.sum() / NCORES) / max(
        1, int(u1_mask.sum() / NCORES)) * 128 / 128)
    # edges per tile-of-128-dsts / 128 = chunks per tile
    avg_c1 = (m1.sum() / NCORES) / (u1_mask.sum() / NCORES / P) / P
    cl1, ch1, T1, packs1 = _choose_caps_and_pack(
        nloc1, d1_loc, s1_l, n1_l, cfg.NPH, avg_c1)
    C1 = cl1 + ch1

    # ---------------- layer-2 structures (dst = variant nodes, owner-shard)
    owner = vni // SH
    CAP = int(-(-max(np.bincount(owner, minlength=NCORES).max(), 1) // P) * P)
    # distinct nodes per core and local slot ids
    dist_nodes, slot_of = [], []
    for q in range(NCORES):
        vq = np.unique(vni[owner == q])
        dist_nodes.append(vq)
        sl = {int(v): i for i, v in enumerate(vq)}
        slot_of.append(sl)
    ND2 = max(len(v) for v in dist_nodes)

    m2e = np.isin(dst_all, vset)
    d2, s2, n2 = dst_all[m2e], srcp_all[m2e], norm[m2e]
    core2 = d2 // SH
    d2_loc, s2_l, n2_l, nloc2 = [], [], [], []
    for q in range(NCORES):
        m = core2 == q
        dd = d2[m]
        sl = slot_of[q]
        d2_loc.append(np.asarray([sl[int(v)] for v in dd], np.int64))
        s2_l.append(s2[m])
        n2_l.append(n2[m])
        nloc2.append(max(len(dist_nodes[q]), 1))
    avg_c2 = (m2e.sum() / NCORES) / max(1.0, ND2 / P) / P
    cl2, ch2, T2, packs2 = _choose_caps_and_pack(
        nloc2, d2_loc, s2_l, n2_l, cfg.NPH, avg_c2)
    C2 = cl2 + ch2
    H2ROWS = T2 * P + P          # +dump tile row space

    # shared weights
    wc1 = np.zeros((cfg.KPAD, H), bf16)
    wc1[:cfg.D_IN] = np.asarray(Wc1, np.float32).astype(bf16)
    wc2 = np.asarray(Wc2, np.float32).astype(bf16)
    wh1 = np.zeros((3 * P, HH), bf16)
    wh1[:H + OH] = np.asarray(Wh1, np.float32).astype(bf16)
    wh2 = np.asarray(Wh2, np.float32).astype(bf16)
    wh3 = np.asarray(Wh3, np.float32).astype(bf16)
    bb1 = np.tile(np.asarray(bc1, np.float32)[None, :], (P, 1))
    bb2 = np.tile(np.asarray(bc2, np.float32)[None, :], (P, 1))
    bh1v = np.asarray(bh1, np.float32).reshape(HH, 1)
    bh2v = np.asarray(bh2, np.float32).reshape(HH // 2, 1)
    bh3v = np.asarray(bh3, np.float32).reshape(1, 1)

    x = np.asarray(x, np.float32)
    wt_b = np.asarray(wt_onehot, np.float32).astype(bf16)
    mut_b = np.asarray(mut_onehot, np.float32).astype(bf16)

    in_maps = []
    out_pos = []
    meta = dict(T1=T1, cl1=cl1, ch1=ch1, T2=T2, cl2=cl2, ch2=ch2, CAP=CAP,
                H2ROWS=H2ROWS)
    for q in range(NCORES):
        tiles, order, starts, lo_mask = packs1[q]
        gidx1, stt1, scat1 = _build_core_arrays(
            T1, cl1, ch1, tiles, order, starts, lo_mask, s1_l[q], n1_l[q],
            cfg.NPH, scat_of=lambda r: r, dump_row=SH)
        tiles2, order2, starts2, lo_mask2 = packs2[q]
        gidx2, stt2, scat2 = _build_core_arrays(
            T2, cl2, ch2, tiles2, order2, starts2, lo_mask2, s2_l[q],
            n2_l[q], cfg.NPH, scat_of=lambda r: 0, dump_row=T2 * P)
        # scat2 maps tile slot -> h2loc row; rebuild using slot layout
        scat2 = np.full((P, T2), T2 * P, np.int32)
        slot_row = np.full(max(len(dist_nodes[q]), 1), T2 * P, np.int64)
        for t, tl in enumerate(tiles2):
            for d, r in enumerate(tl[3]):
                scat2[d, t] = t * P + d
                slot_row[r] = t * P + d

        # per-instance rows for the head (owner order)
        inst_b = np.nonzero(owner == q)[0]          # b indices owned
        nb = len(inst_b)
        hidx = np.zeros(CAP, np.int64)
        for i, b in enumerate(inst_b):
            hidx[i] = slot_row[slot_of[q][int(vni[b])]]
        ohT = np.zeros((OH, CAP), bf16)
        ohT[:20, :nb] = wt_b[inst_b].T
        ohT[20:, :nb] = mut_b[inst_b].T
        out_pos.append(inst_b)

        xT = np.zeros((cfg.KPAD, SHP), bf16)
        xT[:cfg.D_IN, :SH] = x[q * SH:(q + 1) * SH].T.astype(bf16)
        in_maps.append(dict(
            xT=xT, gidx1=gidx1, stt1=stt1, scat1=scat1,
            gidx2=gidx2, stt2=stt2, scat2=scat2,
            hidx=_pack_idx16(hidx), ohT=np.ascontiguousarray(ohT),
            wc1=wc1, wc2=wc2, wh1=wh1, wh2=wh2, wh3=wh3,
            bb1=bb1, bb2=bb2, bh1v=bh1v, bh2v=bh2v, bh3v=bh3v,
        ))
    return in_maps, meta, out_pos


# ------------------------------------------------------------- bass program

def build_program(cfg, meta):
    import concourse.bass as bass
    import concourse.mybir as mybir
    import concourse.tile as tile
    from concourse import bacc
    from concourse.masks import make_identity

    T1, cl1, ch1 = meta["T1"], meta["cl1"], meta["ch1"]
    T2, cl2, ch2 = meta["T2"], meta["cl2"], meta["ch2"]
    CAP, H2ROWS = meta["CAP"], meta["H2ROWS"]
    C1, C2 = cl1 + ch1, cl2 + ch2
    BCH = CAP // P

    nc = bacc.Bacc("TRN2", target_bir_lowering=False, debug=False,
                   num_devices=NCORES, num_swdge_queues=4)
    f32, bfl, i16, i32 = (mybir.dt.float32, mybir.dt.bfloat16,
                          mybir.dt.int16, mybir.dt.int32)

    # I/O
    xT = nc.dram_tensor("xT", [cfg.KPAD, cfg.SHP], bfl, kind="ExternalInput")
    gidx1 = nc.dram_tensor("gidx1", [P, T1 * C1 * 8], i16,
                           kind="ExternalInput")
    stt1 = nc.dram_tensor("stt1", [P, T1 * C1 * P], bfl,
                          kind="ExternalInput")
    scat1 = nc.dram_tensor("scat1", [P, T1], i32, kind="ExternalInput")
    gidx2 = nc.dram_tensor("gidx2", [P, T2 * C2 * 8], i16,
                           kind="ExternalInput")
    stt2 = nc.dram_tensor("stt2", [P, T2 * C2 * P], bfl,
                          kind="ExternalInput")
    scat2 = nc.dram_tensor("scat2", [P, T2], i32, kind="ExternalInput")
    hidx = nc.dram_tensor("hidx", [P, CAP // 16], i16, kind="ExternalInput")
    ohT = nc.dram_tensor("ohT", [OH, CAP], bfl, kind="ExternalInput")
    wc1 = nc.dram_tensor("wc1", [cfg.KPAD, H], bfl, kind="ExternalInput")
    wc2 = nc.dram_tensor("wc2", [H, H], bfl, kind="ExternalInput")
    wh1 = nc.dram_tensor("wh1", [3 * P, HH], bfl, kind="ExternalInput")
    wh2 = nc.dram_tensor("wh2", [HH, HH // 2], bfl, kind="ExternalInput")
    wh3 = nc.dram_tensor("wh3", [HH // 2, 1], bfl, kind="ExternalInput")
    bb1 = nc.dram_tensor("bb1", [P, H], f32, kind="ExternalInput")
    bb2 = nc.dram_tensor("bb2", [P, H], f32, kind="ExternalInput")
    bh1v = nc.dram_tensor("bh1v", [HH, 1], f32, kind="ExternalInput")
    bh2v = nc.dram_tensor("bh2v", [HH // 2, 1], f32, kind="ExternalInput")
    bh3v = nc.dram_tensor("bh3v", [1, 1], f32, kind="ExternalInput")
    out = nc.dram_tensor("out", [1, CAP], f32, kind="ExternalOutput")

    # internal DRAM
    z0in = nc.dram_tensor("z0in", [cfg.SHP, H], bfl, kind="Internal")
    z1in = nc.dram_tensor("z1in", [cfg.SHP, H], bfl, kind="Internal")
    h2loc = nc.dram_tensor("h2loc", [H2ROWS, H], bfl, kind="Internal")
    Z0 = nc.dram_tensor("Z0", [cfg.NP, H], bfl, kind="Internal",
                        addr_space="Shared")
    Z1 = nc.dram_tensor("Z1", [cfg.NP, H], bfl, kind="Internal",
                        addr_space="Shared")
    rg = [list(range(NCORES))]

    with tile.TileContext(nc) as tc:
        with tc.tile_pool(name="const", bufs=1) as const:
            ident = const.tile([P, P], bfl)
            make_identity(nc, ident[:])

            def load(ap, shape, dt):
                t = const.tile(shape, dt, tag=ap.tensor.name)
                nc.sync.dma_start(t[:], ap)
                return t

            wc1_sb = load(wc1.rearrange("(t p) n -> p t n", p=P)[:],
                          [P, cfg.KT, H], bfl)
            wc2_sb = load(wc2.rearrange("(t p) n -> p t n", p=P)[:],
                          [P, 2, H], bfl)
            wh1_sb = load(wh1.rearrange("(t p) n -> p t n", p=P)[:],
                          [P, 3, HH], bfl)
            wh2_sb = load(wh2[:], [HH, HH // 2], bfl)
            wh3_sb = load(wh3[:], [HH // 2, 1], bfl)
            bb1_sb = load(bb1[:], [P, H], f32)
            bb2_sb = load(bb2[:], [P, H], f32)
            bh1_sb = load(bh1v[:], [HH, 1], f32)
            bh2_sb = load(bh2v[:], [HH // 2, 1], f32)
            bh3_sb = load(bh3v[:], [1, 1], f32)
            gidx1_sb = load(gidx1[:], [P, T1 * C1 * 8], i16)
            scat1_sb = load(scat1[:], [P, T1], i32)
            gidx2_sb = load(gidx2[:], [P, T2 * C2 * 8], i16)
            scat2_sb = load(scat2[:], [P, T2], i32)
            hidx_sb = load(hidx[:], [P, CAP // 16], i16)
            ohT_sb = load(ohT[:], [OH, CAP], bfl)

            # zero dump rows of scatter targets
            zpad = const.tile([P, H], bfl)
            nc.any.memset(zpad[:], 0.0)
            npad = cfg.SHP - cfg.SH
            nc.sync.dma_start(z1in[cfg.SH:cfg.SHP, :], zpad[:npad, :])
            nc.sync.dma_start(h2loc[T2 * P:T2 * P + P, :], zpad[:])

            # ---------------- phase A: conv1 linear z0 = x @ Wc1
            MBS = 7
            with tc.tile_pool(name="c1sb", bufs=3) as c1sb, \
                 tc.tile_pool(name="c1ev", bufs=3) as c1ev, \
                 tc.tile_pool(name="c1ps", bufs=MBS + 1, space="PSUM") as c1ps:
                for mb0 in range(0, cfg.MT, MBS):
                    mbn = min(MBS, cfg.MT - mb0)
                    accs = [c1ps.tile([P, H], f32, tag="convacc",
                                      name=f"convacc_{mb0}_{j}")
                            for j in range(mbn)]
                    for kt in range(cfg.KT):
                        slab = c1sb.tile([P, MBS * P], bfl, tag="slab")
                        nc.sync.dma_start(
                            slab[:, :mbn * P],
                            xT[kt * P:(kt + 1) * P, mb0 * P:(mb0 + mbn) * P])
                        for j in range(mbn):
                            nc.tensor.matmul(
                                accs[j][:], lhsT=slab[:, j * P:(j + 1) * P],
                                rhs=wc1_sb[:, kt, :],
                                start=(kt == 0), stop=(kt == cfg.KT - 1))
                    for j in range(mbn):
                        zb = c1ev.tile([P, H], bfl, tag="zev")
                        nc.vector.tensor_copy(zb[:], accs[j][:])
                        r0 = (mb0 + j) * P
                        nc.sync.dma_start(z0in[r0:r0 + P, :], zb[:])

            nc.gpsimd.collective_compute(
                "AllGather", mybir.AluOpType.bypass, replica_groups=rg,
                ins=[z0in[:]], outs=[Z0[:]])

            # ---------------- aggregation layers
            def agg_layer(Z, T, cl, ch, gidx_sb, stt_dram, scat_sb, bias_sb,
                          out_dram, do_conv2):
                C = cl + ch
                with tc.tile_pool(name="agsb", bufs=3) as agsb, \
                     tc.tile_pool(name="agst", bufs=3) as agst, \
                     tc.tile_pool(name="agps", bufs=2, space="PSUM") as agps:
                    Zlo = Z[:cfg.NPH, :]
                    Zhi = Z[cfg.NPH:, :]
                    for t in range(T):
                        msg = agsb.tile([P, C, H], bfl, tag="msg")
                        off = t * C * 8
                        q0 = (2 * t) % 4
                        nc.gpsimd.dma_gather(
                            msg[:, :cl, :], Zlo, gidx_sb[:, off:off + cl * 8],
                            cl * P, cl * P, H, single_packet=False,
                            queue_num=q0)
                        nc.gpsimd.dma_gather(
                            msg[:, cl:, :], Zhi,
                            gidx_sb[:, off + cl * 8:off + C * 8],
                            ch * P, ch * P, H, single_packet=False,
                            queue_num=q0 + 1)
                        st = agst.tile([P, C, P], bfl, tag="st")
                        nc.sync.dma_start(
                            st[:], stt_dram[:, t * C * P:(t + 1) * C * P])
                        acc = agps.tile([P, H], f32, tag="agacc")
                        for c in range(C):
                            nc.tensor.matmul(acc[:], lhsT=st[:, c, :],
                                             rhs=msg[:, c, :],
                                             start=(c == 0), stop=(c == C - 1))
                        hf = agsb.tile([P, H], f32, tag="hf")
                        nc.vector.tensor_tensor(out=hf[:], in0=acc[:],
                                                in1=bias_sb[:],
                                                op=mybir.AluOpType.add)
                        hb = agsb.tile([P, H], bfl, tag="hb")
                        nc.scalar.activation(
                            hb[:], hf[:], mybir.ActivationFunctionType.Relu)
                        if do_conv2:
                            ht = agsb.tile([P, H], bfl, tag="ht")
                            for k in range(2):
                                pt = agps.tile([P, P], bfl, space="PSUM",
                                               tag="pt")
                                nc.tensor.transpose(
                                    pt[:], hb[:, k * P:(k + 1) * P], ident[:])
                                nc.scalar.copy(ht[:, k * P:(k + 1) * P],
                                               pt[:])
                            pz = agps.tile([P, H], f32, tag="pz")
                            for k in range(2):
                                nc.tensor.matmul(
                                    pz[:], lhsT=ht[:, k * P:(k + 1) * P],
                                    rhs=wc2_sb[:, k, :],
                                    start=(k == 0), stop=(k == 1))
                            res = agsb.tile([P, H], bfl, tag="res")
                            nc.vector.tensor_copy(res[:], pz[:])
                        else:
                            res = hb
                        nc.gpsimd.indirect_dma_start(
                            out=out_dram[:],
                            out_offset=bass.IndirectOffsetOnAxis(
                                ap=scat_sb[:, t:t + 1], axis=0),
                            in_=res[:], in_offset=None)

            agg_layer(Z0, T1, cl1, ch1, gidx1_sb, stt1, scat1_sb, bb1_sb,
                      z1in, do_conv2=True)
            nc.gpsimd.collective_compute(
                "AllGather", mybir.AluOpType.bypass, replica_groups=rg,
                ins=[z1in[:]], outs=[Z1[:]])
            agg_layer(Z1, T2, cl2, ch2, gidx2_sb, stt2, scat2_sb, bb2_sb,
                      h2loc, do_conv2=False)

            # ---------------- head (owner-local variants)
            with tc.tile_pool(name="hdsb", bufs=2) as hdsb, \
                 tc.tile_pool(name="hdps", bufs=1, space="PSUM") as hdps:
                g = hdsb.tile([P, BCH, H], bfl, tag="hg")
                nc.gpsimd.dma_gather(
                    g[:], h2loc[:], hidx_sb[:], CAP, CAP, H,
                    single_packet=False)
                zt0 = hdsb.tile([P, CAP], bfl, tag="zt0")
                zt1 = hdsb.tile([P, CAP], bfl, tag="zt1")
                for j in range(BCH):
                    for k in range(2):
                        pt = hdps.tile([P, P], bfl, space="PSUM", tag="hpt")
                        nc.tensor.transpose(
                            pt[:], g[:, j, k * P:(k + 1) * P], ident[:])
                        dstt = zt0 if k == 0 else zt1
                        nc.vector.tensor_copy(
                            dstt[:, j * P:(j + 1) * P], pt[:])
                ph1 = hdps.tile([P, CAP], f32, tag="ph1")
                for c0 in range(0, CAP, 512):
                    cw = min(512, CAP - c0)
                    nc.tensor.matmul(ph1[:, c0:c0 + cw],
                                     lhsT=wh1_sb[:, 0, :],
                                     rhs=zt0[:, c0:c0 + cw],
                                     start=True, stop=False)
                    nc.tensor.matmul(ph1[:, c0:c0 + cw],
                                     lhsT=wh1_sb[:, 1, :],
                                     rhs=zt1[:, c0:c0 + cw],
                                     start=False, stop=False)
                    nc.tensor.matmul(ph1[:, c0:c0 + cw],
                                     lhsT=wh1_sb[:OH, 2, :],
                                     rhs=ohT_sb[:, c0:c0 + cw],
                                     start=False, stop=True)
                a1 = hdsb.tile([P, CAP], bfl, tag="a1")
                nc.scalar.activation(a1[:], ph1[:],
                                     mybir.ActivationFunctionType.Relu,
                                     bias=bh1_sb[:])
                ph2 = hdps.tile([HH // 2, CAP], f32, tag="ph2")
                for c0 in range(0, CAP, 512):
                    cw = min(512, CAP - c0)
                    nc.tensor.matmul(ph2[:, c0:c0 + cw], lhsT=wh2_sb[:],
                                     rhs=a1[:, c0:c0 + cw],
                                     start=True, stop=True)
                a2 = hdsb.tile([HH // 2, CAP], bfl, tag="a2")
                nc.scalar.activation(a2[:], ph2[:],
                                     mybir.ActivationFunctionType.Relu,
                                     bias=bh2_sb[:])
                ph3 = hdps.tile([1, CAP], f32, tag="ph3")
                for c0 in range(0, CAP, 512):
                    cw = min(512, CAP - c0)
                    nc.tensor.matmul(ph3[:, c0:c0 + cw], lhsT=wh3_sb[:],
                                     rhs=a2[:, c0:c0 + cw],
                                     start=True, stop=True)
                osb = hdsb.tile([1, CAP], f32, tag="osb")
                nc.vector.tensor_scalar_add(osb[:], ph3[:], bh3_sb[:, :1])
                nc.sync.dma_start(out[:], osb[:])

    nc.compile()
    return nc


# ------------------------------------------------------------------ driver

_CACHE = {}


def _get_program(cfg, meta):
    key = (cfg.N, cfg.E, cfg.D_IN, cfg.B) + tuple(sorted(meta.items()))
    if key not in _CACHE:
        _CACHE[key] = build_program(cfg, meta)
    return _CACHE[key]


def kernel(**inputs):
    cfg = REAL
    in_maps, meta, out_pos = host_prep(cfg, **inputs)
    nc = _get_program(cfg, meta)
    from concourse import bass_utils
    res = bass_utils.run_bass_kernel_spmd(
        nc, in_maps, core_ids=list(range(NCORES)))
    full = np.zeros(cfg.B, np.float32)
    for q in range(NCORES):
        vals = np.asarray(res.results[q]["out"]).reshape(-1)
        full[out_pos[q]] = vals[:len(out_pos[q])]
    return full.astype(np.float32)


# revision 7
# speedup vs baseline: 3.0438x; 1.3145x over previous
"""Trainium2 Bass kernel for a 2-layer GCN encoder + MLP head (PyG GCNConv).

Strategy (8 NeuronCores, node-parallel), v2:
  - Nodes sharded by contiguous range: core q owns rows [q*SH, (q+1)*SH).
  - conv1 linear (x @ Wc1) computed shard-local on PE (bf16, fp32 PSUM),
    z0 shards AllGather'ed to a replicated table Z0 [NP, 256] bf16.
  - Layer-2 pruning: the head only reads h2 at var_node_idx, so layer 2
    aggregates only over in-edges of the ~4k distinct variant nodes
    (owner-core sharded); no H2 AllGather, the head runs on the owner core
    and the host re-permutes the (value, b-position) pairs.
  - Layer-1 pruning: h1 is only needed at U1 = variants + their in-edge
    sources (~75% of nodes), so layer-1 tiles cover only owned U1 nodes.
  - Aggregation per dst tile: one dma_gather per (tile, half-table) fetches
    edge source rows (SWDGE queues alternate per tile to overlap descriptor
    generation), and PE matmuls accumulate st.T @ msg in PSUM where the
    norm-scaled one-hot st chunks are PREBUILT ON HOST and DMA-loaded
    (frees DVE from 2k tensor_scalar builds which contend with SWDGE).
  - dma_gather indices are int16 so the Z table is split lo/hi.
  - conv2 fused into layer-1 tile epilogue (PE transpose + matmul).
All heavy compute is bf16 with fp32 PSUM accumulation.
"""
import sys

for _p in ("/opt/trn_rl_repo",):
    if _p not in sys.path:
        sys.path.insert(0, _p)

import numpy as np
import ml_dtypes

bf16 = ml_dtypes.bfloat16

P = 128
H = 256          # gcn hidden width (fixed)
HH = 128         # head hidden width (fixed)
OH = 40          # wt_onehot + mut_onehot width (fixed)
NCORES = 8


class Cfg:
    def __init__(self, N, E, D_IN, B):
        self.N, self.E, self.D_IN, self.B = N, E, D_IN, B
        assert N % NCORES == 0
        self.SH = N // NCORES                      # real rows per shard
        shp = -(-self.SH // P) * P
        if shp == self.SH:
            shp += P                               # need >=1 dump row
        self.SHP = shp                             # padded rows per shard
        self.NP = NCORES * self.SHP                # padded global rows
        assert self.NP % 2 == 0
        self.NPH = self.NP // 2                    # half-table rows (int16 idx)
        assert self.NPH < 32768
        self.KT = -(-D_IN // P)                    # k tiles for conv1
        self.KPAD = self.KT * P
        self.MT = self.SHP // P                    # m tiles per shard
        self.B = B


REAL = Cfg(N=50000, E=800000, D_IN=1281, B=4096)


# ---------------------------------------------------------------- host prep

def _pack_idx16(seq):
    """idx sequence [n] -> wrapped-16 + replicated layout [128, n//16] int16."""
    n = seq.shape[0]
    assert n % 16 == 0
    a = seq.reshape(n // 16, 16).T.astype(np.int16)
    return np.tile(a, (8, 1))


def _pack_core(nloc, cl, ch, d_loc, srcp, nv, nph):
    """Bin-pack edges into tiles (<=128 dst nodes, <=cl*128 lo edges,
    <=ch*128 hi edges). d_loc in [0, nloc)."""
    order = np.argsort(d_loc, kind="stable")
    d_s = d_loc[order]
    counts = np.bincount(d_s, minlength=nloc)
    starts = np.zeros(nloc + 1, np.int64)
    np.cumsum(counts, out=starts[1:])
    lo_mask = srcp[order] < nph
    klo = np.zeros(nloc, np.int64)
    np.add.at(klo, d_s[lo_mask], 1)
    khi = counts - klo

    node_order = np.argsort(-counts, kind="stable")
    cap_l, cap_h = cl * P, ch * P
    tiles = []  # [n_nodes, lo_cnt, hi_cnt, node_list]
    for r in node_order:
        if counts[r] == 0 and klo[r] == 0:
            # nodes with no edges still need a slot if they are real dsts
            # (can't happen: self loops guarantee >=1 edge)
            continue
        kl, kh = klo[r], khi[r]
        placed = False
        for t in tiles:
            if t[0] < P and t[1] + kl <= cap_l and t[2] + kh <= cap_h:
                t[0] += 1
                t[1] += kl
                t[2] += kh
                t[3].append(r)
                placed = True
                break
        if not placed:
            tiles.append([1, kl, kh, [r]])
    return tiles, order, starts, lo_mask


def _build_core_arrays(T, cl, ch, tiles, order, starts, lo_mask, srcp, nv,
                       nph, scat_of, dump_row):
    """Build gidx/stt/scat arrays for one core and one layer.
    stt is the norm-scaled one-hot, host-prebuilt: [128, T*C*128] bf16."""
    C = cl + ch
    gidx_seq = np.zeros(T * C * P, np.int64)
    stt = np.zeros((P, T * C * P), bf16)
    scat = np.full((P, T), dump_row, np.int32)

    for t, tl in enumerate(tiles):
        lo_idx, lo_d, lo_n = [], [], []
        hi_idx, hi_d, hi_n = [], [], []
        for d, r in enumerate(tl[3]):
            scat[d, t] = scat_of(r)
            es = order[starts[r]:starts[r + 1]]
            lm = lo_mask[starts[r]:starts[r + 1]]
            sp = srcp[es]
            nn = nv[es]
            lo_idx.extend(sp[lm].tolist())
            lo_d.extend([d] * int(lm.sum()))
            lo_n.extend(nn[lm].tolist())
            hm = ~lm
            hi_idx.extend((sp[hm] - nph).tolist())
            hi_d.extend([d] * int(hm.sum()))
            hi_n.extend(nn[hm].tolist())
        npad_l = cl * P - len(lo_idx)
        npad_h = ch * P - len(hi_idx)
        assert npad_l >= 0 and npad_h >= 0
        seq_idx = lo_idx + [0] * npad_l + hi_idx + [0] * npad_h
        seq_d = np.asarray(lo_d + [-1] * npad_l + hi_d + [-1] * npad_h,
                           np.int64)
        seq_n = np.asarray(lo_n + [0.0] * npad_l + hi_n + [0.0] * npad_h,
                           np.float32)
        base = t * C * P
        gidx_seq[base:base + C * P] = seq_idx
        # slot j = c*128 + p  ->  stt[p, base + c*128 + d] = norm
        valid = seq_d >= 0
        jj = np.nonzero(valid)[0]
        pp = jj % P
        cc = jj // P
        stt[pp, base + cc * P + seq_d[jj]] = seq_n[jj].astype(bf16)

    cols = []
    for t in range(T):
        base = t * C * P
        cols.append(_pack_idx16(gidx_seq[base:base + cl * P]))
        cols.append(_pack_idx16(gidx_seq[base + cl * P:base + C * P]))
    gidx = np.concatenate(cols, axis=1)  # [128, T*C*8]
    return gidx, stt, scat


def _choose_caps_and_pack(nloc_list, d_loc_list, srcp_list, nv_list, nph,
                          avg_c):
    """Pick (cl, ch) caps minimizing T*(cl+ch) over all cores; return packs."""
    base = max(1, int(np.ceil(avg_c / 2)))
    cands = [(base, base), (base + 1, base + 1), (base, base + 1),
             (base + 1, base), (base + 2, base + 2)]
    best = None
    for (cl, ch) in cands:
        packs, Ts = [], []
        for q in range(NCORES):
            pk = _pack_core(nloc_list[q], cl, ch, d_loc_list[q],
                            srcp_list[q], nv_list[q], nph)
            packs.append(pk)
            Ts.append(len(pk[0]))
        T_need = max(Ts)
        cost = T_need * (cl + ch)
        if best is None or cost < best[0]:
            best = (cost, cl, ch, T_need, packs)
    _, cl, ch, T, packs = best
    return cl, ch, T, packs


def host_prep(cfg, x, wt_onehot, mut_onehot, Wc1, bc1, Wc2, bc2,
              Wh1, bh1, Wh2, bh2, Wh3, bh3, edge_index, var_node_idx):
    N, E, SH, SHP = cfg.N, cfg.E, cfg.SH, cfg.SHP
    src = np.asarray(edge_index[0], np.int64)
    dst = np.asarray(edge_index[1], np.int64)
    loop = np.arange(N, dtype=np.int64)
    src_all = np.concatenate([src, loop])
    dst_all = np.concatenate([dst, loop])
    deg = np.bincount(dst_all, minlength=N).astype(np.float32)
    dinv = np.where(deg > 0, 1.0 / np.sqrt(np.maximum(deg, 1.0)),
                    0.0).astype(np.float32)
    norm = (dinv[src_all] * dinv[dst_all]).astype(np.float32)
    srcp_all = (src_all // SH) * SHP + (src_all % SH)

    vni = np.asarray(var_node_idx, np.int64)
    vset = np.unique(vni)
    # U1: nodes whose h1 is needed = variant nodes + sources of their in-edges
    m2 = np.isin(dst_all, vset)
    u1 = np.unique(np.concatenate([vset, src_all[m2]]))
    u1_mask = np.zeros(N, bool)
    u1_mask[u1] = True

    # ---------------- layer-1 structures (dst in owned U1)
    m1 = u1_mask[dst_all]
    d1, s1, n1 = dst_all[m1], srcp_all[m1], norm[m1]
    core1 = d1 // SH
    d1_loc, s1_l, n1_l, nloc1 = [], [], [], []
    for q in range(NCORES):
        m = core1 == q
        d1_loc.append(d1[m] - q * SH)
        s1_l.append(s1[m])
        n1_l.append(n1[m])
        nloc1.append(SH)
    avg_c1 = (m1.sum() / NCORES) / (SH * 0.754) * P / P  # rough
    avg_c1 = max(2.0, (m1.sum() / NCORES) / max(
        1, int(u1_mask.sum() / NCORES)) * 128 / 128)
    # edges per tile-of-128-dsts / 128 = chunks per tile
    avg_c1 = (m1.sum() / NCORES) / (u1_mask.sum() / NCORES / P) / P
    cl1, ch1, T1, packs1 = _choose_caps_and_pack(
        nloc1, d1_loc, s1_l, n1_l, cfg.NPH, avg_c1)
    C1 = cl1 + ch1

    # L1 dense output layout: core q's tile t slot d -> global row
    # q*T1*128 + t*128 + d  (z1 written densely, no indirect scatter)
    T1P = T1 * P
    NP1 = NCORES * T1P
    assert NP1 % 2 == 0 and NP1 // 2 < 32768
    NPH1 = NP1 // 2
    srcp1_of = np.full(N, -1, np.int64)   # node -> dense z1 row
    for q in range(NCORES):
        tiles, _, _, _ = packs1[q]
        for t, tl in enumerate(tiles):
            for d, r in enumerate(tl[3]):
                srcp1_of[q * SH + r] = q * T1P + t * P + d

    # ---------------- layer-2 structures (dst = variant nodes, owner-shard)
    owner = vni // SH
    CAP = int(-(-max(np.bincount(owner, minlength=NCORES).max(), 1) // P) * P)
    # distinct nodes per core and local slot ids
    dist_nodes, slot_of = [], []
    for q in range(NCORES):
        vq = np.unique(vni[owner == q])
        dist_nodes.append(vq)
        sl = {int(v): i for i, v in enumerate(vq)}
        slot_of.append(sl)
    ND2 = max(len(v) for v in dist_nodes)

    m2e = np.isin(dst_all, vset)
    s2_dense = srcp1_of[src_all[m2e]]
    assert (s2_dense >= 0).all(), "L2 source not computed in L1"
    d2, s2, n2 = dst_all[m2e], s2_dense, norm[m2e]
    core2 = d2 // SH
    d2_loc, s2_l, n2_l, nloc2 = [], [], [], []
    for q in range(NCORES):
        m = core2 == q
        dd = d2[m]
        sl = slot_of[q]
        d2_loc.append(np.asarray([sl[int(v)] for v in dd], np.int64))
        s2_l.append(s2[m])
        n2_l.append(n2[m])
        nloc2.append(max(len(dist_nodes[q]), 1))
    avg_c2 = (m2e.sum() / NCORES) / max(1.0, ND2 / P) / P
    cl2, ch2, T2, packs2 = _choose_caps_and_pack(
        nloc2, d2_loc, s2_l, n2_l, NPH1, avg_c2)
    C2 = cl2 + ch2
    H2ROWS = T2 * P + P          # +dump tile row space

    # shared weights
    wc1 = np.zeros((cfg.KPAD, H), bf16)
    wc1[:cfg.D_IN] = np.asarray(Wc1, np.float32).astype(bf16)
    wc2 = np.asarray(Wc2, np.float32).astype(bf16)
    wh1 = np.zeros((3 * P, HH), bf16)
    wh1[:H + OH] = np.asarray(Wh1, np.float32).astype(bf16)
    wh2 = np.asarray(Wh2, np.float32).astype(bf16)
    wh3 = np.asarray(Wh3, np.float32).astype(bf16)
    bb1 = np.tile(np.asarray(bc1, np.float32)[None, :], (P, 1))
    bb2 = np.tile(np.asarray(bc2, np.float32)[None, :], (P, 1))
    bh1v = np.asarray(bh1, np.float32).reshape(HH, 1)
    bh2v = np.asarray(bh2, np.float32).reshape(HH // 2, 1)
    bh3v = np.asarray(bh3, np.float32).reshape(1, 1)

    x = np.asarray(x, np.float32)
    wt_b = np.asarray(wt_onehot, np.float32).astype(bf16)
    mut_b = np.asarray(mut_onehot, np.float32).astype(bf16)

    in_maps = []
    out_pos = []
    meta = dict(T1=T1, cl1=cl1, ch1=ch1, T2=T2, cl2=cl2, ch2=ch2, CAP=CAP,
                H2ROWS=H2ROWS, NP1=NP1, NPH1=NPH1)
    for q in range(NCORES):
        tiles, order, starts, lo_mask = packs1[q]
        gidx1, stt1, scat1 = _build_core_arrays(
            T1, cl1, ch1, tiles, order, starts, lo_mask, s1_l[q], n1_l[q],
            cfg.NPH, scat_of=lambda r: r, dump_row=SH)
        tiles2, order2, starts2, lo_mask2 = packs2[q]
        gidx2, stt2, scat2 = _build_core_arrays(
            T2, cl2, ch2, tiles2, order2, starts2, lo_mask2, s2_l[q],
            n2_l[q], NPH1, scat_of=lambda r: 0, dump_row=T2 * P)
        # scat2 maps tile slot -> h2loc row; rebuild using slot layout
        scat2 = np.full((P, T2), T2 * P, np.int32)
        slot_row = np.full(max(len(dist_nodes[q]), 1), T2 * P, np.int64)
        for t, tl in enumerate(tiles2):
            for d, r in enumerate(tl[3]):
                scat2[d, t] = t * P + d
                slot_row[r] = t * P + d

        # per-instance rows for the head (owner order)
        inst_b = np.nonzero(owner == q)[0]          # b indices owned
        nb = len(inst_b)
        hidx = np.zeros(CAP, np.int64)
        for i, b in enumerate(inst_b):
            hidx[i] = slot_row[slot_of[q][int(vni[b])]]
        ohT = np.zeros((OH, CAP), bf16)
        ohT[:20, :nb] = wt_b[inst_b].T
        ohT[20:, :nb] = mut_b[inst_b].T
        out_pos.append(inst_b)

        xT = np.zeros((cfg.KPAD, SHP), bf16)
        xT[:cfg.D_IN, :SH] = x[q * SH:(q + 1) * SH].T.astype(bf16)
        in_maps.append(dict(
            xT=xT, gidx1=gidx1, stt1=stt1,
            gidx2=gidx2, stt2=stt2,
            hidx=_pack_idx16(hidx), ohT=np.ascontiguousarray(ohT),
            wc1=wc1, wc2=wc2, wh1=wh1, wh2=wh2, wh3=wh3,
            bb1=bb1, bb2=bb2, bh1v=bh1v, bh2v=bh2v, bh3v=bh3v,
        ))
    return in_maps, meta, out_pos


# ------------------------------------------------------------- bass program

def build_program(cfg, meta):
    import concourse.bass as bass
    import concourse.mybir as mybir
    import concourse.tile as tile
    from concourse import bacc
    from concourse.masks import make_identity

    T1, cl1, ch1 = meta["T1"], meta["cl1"], meta["ch1"]
    T2, cl2, ch2 = meta["T2"], meta["cl2"], meta["ch2"]
    CAP, H2ROWS = meta["CAP"], meta["H2ROWS"]
    C1, C2 = cl1 + ch1, cl2 + ch2
    BCH = CAP // P

    nc = bacc.Bacc("TRN2", target_bir_lowering=False, debug=False,
                   num_devices=NCORES, num_swdge_queues=4)
    f32, bfl, i16, i32 = (mybir.dt.float32, mybir.dt.bfloat16,
                          mybir.dt.int16, mybir.dt.int32)

    # I/O
    xT = nc.dram_tensor("xT", [cfg.KPAD, cfg.SHP], bfl, kind="ExternalInput")
    gidx1 = nc.dram_tensor("gidx1", [P, T1 * C1 * 8], i16,
                           kind="ExternalInput")
    stt1 = nc.dram_tensor("stt1", [P, T1 * C1 * P], bfl,
                          kind="ExternalInput")
    gidx2 = nc.dram_tensor("gidx2", [P, T2 * C2 * 8], i16,
                           kind="ExternalInput")
    stt2 = nc.dram_tensor("stt2", [P, T2 * C2 * P], bfl,
                          kind="ExternalInput")
    hidx = nc.dram_tensor("hidx", [P, CAP // 16], i16, kind="ExternalInput")
    ohT = nc.dram_tensor("ohT", [OH, CAP], bfl, kind="ExternalInput")
    wc1 = nc.dram_tensor("wc1", [cfg.KPAD, H], bfl, kind="ExternalInput")
    wc2 = nc.dram_tensor("wc2", [H, H], bfl, kind="ExternalInput")
    wh1 = nc.dram_tensor("wh1", [3 * P, HH], bfl, kind="ExternalInput")
    wh2 = nc.dram_tensor("wh2", [HH, HH // 2], bfl, kind="ExternalInput")
    wh3 = nc.dram_tensor("wh3", [HH // 2, 1], bfl, kind="ExternalInput")
    bb1 = nc.dram_tensor("bb1", [P, H], f32, kind="ExternalInput")
    bb2 = nc.dram_tensor("bb2", [P, H], f32, kind="ExternalInput")
    bh1v = nc.dram_tensor("bh1v", [HH, 1], f32, kind="ExternalInput")
    bh2v = nc.dram_tensor("bh2v", [HH // 2, 1], f32, kind="ExternalInput")
    bh3v = nc.dram_tensor("bh3v", [1, 1], f32, kind="ExternalInput")
    out = nc.dram_tensor("out", [1, CAP], f32, kind="ExternalOutput")

    T1P = T1 * P
    NP1 = meta["NP1"]
    # internal DRAM
    z0in = nc.dram_tensor("z0in", [cfg.SHP, H], bfl, kind="Internal")
    z1d = nc.dram_tensor("z1d", [T1P, H], bfl, kind="Internal")
    h2loc = nc.dram_tensor("h2loc", [H2ROWS, H], bfl, kind="Internal")
    Z0 = nc.dram_tensor("Z0", [cfg.NP, H], bfl, kind="Internal",
                        addr_space="Shared")
    Z1 = nc.dram_tensor("Z1", [NP1, H], bfl, kind="Internal",
                        addr_space="Shared")
    rg = [list(range(NCORES))]

    with tile.TileContext(nc) as tc:
        with tc.tile_pool(name="const", bufs=1) as const:
            ident = const.tile([P, P], bfl)
            make_identity(nc, ident[:])

            def load(ap, shape, dt):
                t = const.tile(shape, dt, tag=ap.tensor.name)
                nc.sync.dma_start(t[:], ap)
                return t

            wc1_sb = load(wc1.rearrange("(t p) n -> p t n", p=P)[:],
                          [P, cfg.KT, H], bfl)
            wc2_sb = load(wc2.rearrange("(t p) n -> p t n", p=P)[:],
                          [P, 2, H], bfl)
            wh1_sb = load(wh1.rearrange("(t p) n -> p t n", p=P)[:],
                          [P, 3, HH], bfl)
            wh2_sb = load(wh2[:], [HH, HH // 2], bfl)
            wh3_sb = load(wh3[:], [HH // 2, 1], bfl)
            bb1_sb = load(bb1[:], [P, H], f32)
            bb2_sb = load(bb2[:], [P, H], f32)
            bh1_sb = load(bh1v[:], [HH, 1], f32)
            bh2_sb = load(bh2v[:], [HH // 2, 1], f32)
            bh3_sb = load(bh3v[:], [1, 1], f32)
            gidx1_sb = load(gidx1[:], [P, T1 * C1 * 8], i16)
            gidx2_sb = load(gidx2[:], [P, T2 * C2 * 8], i16)
            hidx_sb = load(hidx[:], [P, CAP // 16], i16)
            ohT_sb = load(ohT[:], [OH, CAP], bfl)


            # ---------------- phase A: conv1 linear z0 = x @ Wc1
            MBS = 7
            with tc.tile_pool(name="c1sb", bufs=3) as c1sb, \
                 tc.tile_pool(name="c1ev", bufs=3) as c1ev, \
                 tc.tile_pool(name="c1ps", bufs=MBS + 1, space="PSUM") as c1ps:
                for mb0 in range(0, cfg.MT, MBS):
                    mbn = min(MBS, cfg.MT - mb0)
                    accs = [c1ps.tile([P, H], f32, tag="convacc",
                                      name=f"convacc_{mb0}_{j}")
                            for j in range(mbn)]
                    slab = c1sb.tile([P, cfg.KT, MBS * P], bfl, tag="slab")
                    nc.sync.dma_start(
                        slab[:, :, :mbn * P],
                        xT.rearrange("(t p) n -> p t n", p=P)[
                            :, :, mb0 * P:(mb0 + mbn) * P])
                    for kt in range(cfg.KT):
                        for j in range(mbn):
                            nc.tensor.matmul(
                                accs[j][:],
                                lhsT=slab[:, kt, j * P:(j + 1) * P],
                                rhs=wc1_sb[:, kt, :],
                                start=(kt == 0), stop=(kt == cfg.KT - 1))
                    for j in range(mbn):
                        zb = c1ev.tile([P, H], bfl, tag="zev")
                        nc.vector.tensor_copy(zb[:], accs[j][:])
                        r0 = (mb0 + j) * P
                        nc.sync.dma_start(z0in[r0:r0 + P, :], zb[:])

            nc.gpsimd.collective_compute(
                "AllGather", mybir.AluOpType.bypass, replica_groups=rg,
                ins=[z0in[:]], outs=[Z0[:]])

            # ---------------- aggregation layers
            def agg_layer(Z, nph, T, cl, ch, gidx_sb, stt_dram, bias_sb,
                          out_dram, do_conv2):
                C = cl + ch
                with tc.tile_pool(name="agsb", bufs=3) as agsb, \
                     tc.tile_pool(name="agst", bufs=3) as agst, \
                     tc.tile_pool(name="agps", bufs=2, space="PSUM") as agps:
                    Zlo = Z[:nph, :]
                    Zhi = Z[nph:, :]
                    for t in range(T):
                        msg = agsb.tile([P, C, H], bfl, tag="msg")
                        off = t * C * 8
                        nc.gpsimd.dma_gather(
                            msg[:, :cl, :], Zlo, gidx_sb[:, off:off + cl * 8],
                            cl * P, cl * P, H, single_packet=False,
                            queue_num=0)
                        nc.gpsimd.dma_gather(
                            msg[:, cl:, :], Zhi,
                            gidx_sb[:, off + cl * 8:off + C * 8],
                            ch * P, ch * P, H, single_packet=False,
                            queue_num=1)
                        st = agst.tile([P, C, P], bfl, tag="st")
                        nc.sync.dma_start(
                            st[:], stt_dram[:, t * C * P:(t + 1) * C * P])
                        acc = agps.tile([P, H], f32, tag="agacc")
                        for c in range(C):
                            nc.tensor.matmul(acc[:], lhsT=st[:, c, :],
                                             rhs=msg[:, c, :],
                                             start=(c == 0), stop=(c == C - 1))
                        hf = agsb.tile([P, H], f32, tag="hf")
                        nc.vector.tensor_tensor(out=hf[:], in0=acc[:],
                                                in1=bias_sb[:],
                                                op=mybir.AluOpType.add)
                        hb = agsb.tile([P, H], bfl, tag="hb")
                        nc.scalar.activation(
                            hb[:], hf[:], mybir.ActivationFunctionType.Relu)
                        if do_conv2:
                            ht = agsb.tile([P, H], bfl, tag="ht")
                            for k in range(2):
                                pt = agps.tile([P, P], bfl, space="PSUM",
                                               tag="pt")
                                nc.tensor.transpose(
                                    pt[:], hb[:, k * P:(k + 1) * P], ident[:])
                                nc.scalar.copy(ht[:, k * P:(k + 1) * P],
                                               pt[:])
                            pz = agps.tile([P, H], f32, tag="pz")
                            for k in range(2):
                                nc.tensor.matmul(
                                    pz[:], lhsT=ht[:, k * P:(k + 1) * P],
                                    rhs=wc2_sb[:, k, :],
                                    start=(k == 0), stop=(k == 1))
                            res = agsb.tile([P, H], bfl, tag="res")
                            nc.vector.tensor_copy(res[:], pz[:])
                        else:
                            res = hb
                        nc.sync.dma_start(out_dram[t * P:(t + 1) * P, :],
                                          res[:])

            agg_layer(Z0, cfg.NPH, T1, cl1, ch1, gidx1_sb, stt1, bb1_sb,
                      z1d, do_conv2=True)
            nc.gpsimd.collective_compute(
                "AllGather", mybir.AluOpType.bypass, replica_groups=rg,
                ins=[z1d[:]], outs=[Z1[:]])
            agg_layer(Z1, meta["NPH1"], T2, cl2, ch2, gidx2_sb, stt2, bb2_sb,
                      h2loc, do_conv2=False)

            # ---------------- head (owner-local variants)
            with tc.tile_pool(name="hdsb", bufs=2) as hdsb, \
                 tc.tile_pool(name="hdps", bufs=1, space="PSUM") as hdps:
                g = hdsb.tile([P, BCH, H], bfl, tag="hg")
                nc.gpsimd.dma_gather(
                    g[:], h2loc[:], hidx_sb[:], CAP, CAP, H,
                    single_packet=False)
                zt0 = hdsb.tile([P, CAP], bfl, tag="zt0")
                zt1 = hdsb.tile([P, CAP], bfl, tag="zt1")
                for j in range(BCH):
                    for k in range(2):
                        pt = hdps.tile([P, P], bfl, space="PSUM", tag="hpt")
                        nc.tensor.transpose(
                            pt[:], g[:, j, k * P:(k + 1) * P], ident[:])
                        dstt = zt0 if k == 0 else zt1
                        nc.vector.tensor_copy(
                            dstt[:, j * P:(j + 1) * P], pt[:])
                ph1 = hdps.tile([P, CAP], f32, tag="ph1")
                for c0 in range(0, CAP, 512):
                    cw = min(512, CAP - c0)
                    nc.tensor.matmul(ph1[:, c0:c0 + cw],
                                     lhsT=wh1_sb[:, 0, :],
                                     rhs=zt0[:, c0:c0 + cw],
                                     start=True, stop=False)
                    nc.tensor.matmul(ph1[:, c0:c0 + cw],
                                     lhsT=wh1_sb[:, 1, :],
                                     rhs=zt1[:, c0:c0 + cw],
                                     start=False, stop=False)
                    nc.tensor.matmul(ph1[:, c0:c0 + cw],
                                     lhsT=wh1_sb[:OH, 2, :],
                                     rhs=ohT_sb[:, c0:c0 + cw],
                                     start=False, stop=True)
                a1 = hdsb.tile([P, CAP], bfl, tag="a1")
                nc.scalar.activation(a1[:], ph1[:],
                                     mybir.ActivationFunctionType.Relu,
                                     bias=bh1_sb[:])
                ph2 = hdps.tile([HH // 2, CAP], f32, tag="ph2")
                for c0 in range(0, CAP, 512):
                    cw = min(512, CAP - c0)
                    nc.tensor.matmul(ph2[:, c0:c0 + cw], lhsT=wh2_sb[:],
                                     rhs=a1[:, c0:c0 + cw],
                                     start=True, stop=True)
                a2 = hdsb.tile([HH // 2, CAP], bfl, tag="a2")
                nc.scalar.activation(a2[:], ph2[:],
                                     mybir.ActivationFunctionType.Relu,
                                     bias=bh2_sb[:])
                ph3 = hdps.tile([1, CAP], f32, tag="ph3")
                for c0 in range(0, CAP, 512):
                    cw = min(512, CAP - c0)
                    nc.tensor.matmul(ph3[:, c0:c0 + cw], lhsT=wh3_sb[:],
                                     rhs=a2[:, c0:c0 + cw],
                                     start=True, stop=True)
                osb = hdsb.tile([1, CAP], f32, tag="osb")
                nc.vector.tensor_scalar_add(osb[:], ph3[:], bh3_sb[:, :1])
                nc.sync.dma_start(out[:], osb[:])

    nc.compile()
    return nc


# ------------------------------------------------------------------ driver

_CACHE = {}


def _get_program(cfg, meta):
    key = (cfg.N, cfg.E, cfg.D_IN, cfg.B) + tuple(sorted(meta.items()))
    if key not in _CACHE:
        _CACHE[key] = build_program(cfg, meta)
    return _CACHE[key]


def kernel(**inputs):
    cfg = REAL
    in_maps, meta, out_pos = host_prep(cfg, **inputs)
    nc = _get_program(cfg, meta)
    from concourse import bass_utils
    res = bass_utils.run_bass_kernel_spmd(
        nc, in_maps, core_ids=list(range(NCORES)))
    full = np.zeros(cfg.B, np.float32)
    for q in range(NCORES):
        vals = np.asarray(res.results[q]["out"]).reshape(-1)
        full[out_pos[q]] = vals[:len(out_pos[q])]
    return full.astype(np.float32)
